# revision 3
# baseline (speedup 1.0000x reference)
"""DiT attention kernel for 8 Trainium2 NeuronCores (v2).

Sharding: tensor-parallel over head groups (4 groups of 4 heads) x
data-parallel over batch (2) = 8 cores. Each core computes, for its batch b
and head group g (heads 4g..4g+3):
    Q^T/K^T = (Wq/Wk slice)^T x^T in [gc, seq] layout (gc = 256 local dims)
    V in natural [seq, gc] layout (no PE transpose needed)
    partial rotary on global head 0 (cores with g==0; others get cos=1/sin=0)
    per head: S^T = K Q^T, P^T = exp(S^T/8), O^T = V_aug^T P^T where V_aug
    carries a ones column per head giving softmax denominators as an extra
    output row; normalize via reciprocal + sbuf broadcast-DMA; Wo partial
    product in natural [seq, dim] layout, summed on host across head groups.

Everything flows through the PE in bf16 (fp32 PSUM accumulation); inputs are
cast host-side. Emission is manually software-pipelined: exp (ACT) is the
pace-setter, all other PE work is interleaved between score matmuls in small
chunks to avoid head-of-line blocking.

vstore column layout per key block tile [128, 260]: head h occupies cols
[65h, 65h+64) with a ones column at 65h+64, so each head's PV output lands
at partitions 0..64 with the softmax denominator in row 64. Odd heads'
normalized output is staged through a tmp tile and DMA-shifted to otst rows
64..128 (DVE cannot write partition base 64 from base-0 inputs).
"""

import os
import sys

if "/opt/trn_rl_repo" not in sys.path:
    sys.path.insert(0, "/opt/trn_rl_repo")



from collections import deque
from contextlib import ExitStack

import numpy as np

import concourse.bass as bass  # noqa: F401  (bass must import before bacc)
import concourse.mybir as mybir
import concourse.tile as tile
from concourse import bacc
from concourse.bass_utils import run_bass_kernel_spmd

F32 = mybir.dt.float32
BF16 = mybir.dt.bfloat16
EXP_FN = mybir.ActivationFunctionType.Exp

B, S, DIM, HEADS, HEAD_DIM = 2, 2048, 1024, 16, 64
N_CORES = 8
TP = 4                      # head groups
GH = HEADS // TP            # heads per core (4)
GC = GH * HEAD_DIM          # cols per core slice (256)
QG = 1024                   # query group width
PT_BUFS = 38                # pts ring depth (~2 iterations + slack)


def _emit_body(nc, tc, ctx, d, inplace_rope=True):
    consts = ctx.enter_context(tc.tile_pool(name="consts", bufs=1))

    ones4 = consts.tile([128, 4], F32, name="ones4", tag="ones4")
    nc.vector.memset(ones4[:], 1.0)

    bq_sb = consts.tile([128, 2], F32, name="bq", tag="bq")
    bk_sb = consts.tile([128, 2], F32, name="bk", tag="bk")
    bvrep = consts.tile([128, GC], F32, name="bvrep", tag="bvrep")
    nc.sync.dma_start(out=bq_sb[:], in_=d["bq2"][:, :])
    nc.sync.dma_start(out=bk_sb[:], in_=d["bk2"][:, :])
    nc.sync.dma_start(out=bvrep[:], in_=d["bvrow"][:].to_broadcast([128, GC]))

    qt = [consts.tile([128, S], BF16, name=f"qt{i}", tag=f"qt{i}") for i in range(2)]
    kt = [consts.tile([128, S], BF16, name=f"kt{i}", tag=f"kt{i}") for i in range(2)]
    if not inplace_rope:
        qtr = consts.tile([64, S], BF16, name="qtr", tag="qtr")
        ktr = consts.tile([64, S], BF16, name="ktr", tag="ktr")
    vstore = [consts.tile([128, 260], BF16, name=f"vs{i}", tag=f"vs{i}") for i in range(16)]
    # normalized O^T, heads stacked per pair p: rows 0:64 head 2p, 64:128 head 2p+1
    otst = [consts.tile([128, S], BF16, name=f"ot{i}", tag=f"ot{i}") for i in range(2)]

    cos_sb = consts.tile([64, S], BF16, name="cos", tag="cos")
    sin_sb = consts.tile([64, S], BF16, name="sin", tag="sin")
    rotm_sb = consts.tile([64, 64], BF16, name="rotm", tag="rotm")
    wo_sb = [consts.tile([128, DIM], BF16, name=f"wo{k}", tag=f"wo{k}") for k in range(2)]

    # ones columns of vstore: col 65h+64 for each head h
    for sc in range(16):
        v3 = vstore[sc][:].rearrange("p (h c) -> p h c", h=4)
        nc.vector.tensor_copy(v3[:, :, 64:65], ones4[:].rearrange("p (h c) -> p h c", h=4))

    xw = ctx.enter_context(tc.tile_pool(name="xw", bufs=1))
    wsp = ctx.enter_context(tc.tile_pool(name="wstream", bufs=24))
    ptp = ctx.enter_context(tc.tile_pool(name="ptp", bufs=PT_BUFS))
    nrm = ctx.enter_context(tc.tile_pool(name="nrm", bufs=3))
    rp = ctx.enter_context(tc.tile_pool(name="rope", bufs=2))
    dscr = ctx.enter_context(tc.tile_pool(name="dscr", bufs=4, space="DRAM"))

    # psum: st 2x2 banks + gp 2x1 + otp 2x1 = 8 banks
    stp = ctx.enter_context(tc.tile_pool(name="stp", bufs=2, space="PSUM"))
    gpp = ctx.enter_context(tc.tile_pool(name="gpp", bufs=2, space="PSUM"))
    otp = ctx.enter_context(tc.tile_pool(name="otp", bufs=2, space="PSUM"))

    xt = [xw.tile([128, S], BF16, name=f"xt{k}", tag=f"xt{k}") for k in range(8)]
    wq_t = [wsp.tile([128, GC], BF16, name="w", tag="w") for _ in range(8)]
    wk_t = [wsp.tile([128, GC], BF16, name="w", tag="w") for _ in range(8)]
    wv_t = [wsp.tile([128, GC], BF16, name="w", tag="w") for _ in range(8)]

    # input DMAs, ordered by need
    for k in range(8):
        nc.sync.dma_start(out=wq_t[k][:], in_=d["wq"][128 * k : 128 * (k + 1), :])
        nc.sync.dma_start(out=xt[k][:, 0:QG], in_=d["xT"][128 * k : 128 * (k + 1), 0:QG])
    for k in range(8):
        nc.sync.dma_start(out=wk_t[k][:], in_=d["wk"][128 * k : 128 * (k + 1), :])
    nc.sync.dma_start(out=cos_sb[:], in_=d["cosT"][:, :])
    nc.sync.dma_start(out=sin_sb[:], in_=d["sinT"][:, :])
    nc.sync.dma_start(out=rotm_sb[:], in_=d["rotm"][:, :])

    def load_xtb():
        # second halves of x^T; deferred past the pre-stream chains so
        # whole-tile deps don't gate them on these DMAs
        for k in range(8):
            nc.sync.dma_start(
                out=xt[k][:, QG:S], in_=d["xT"][128 * k : 128 * (k + 1), QG:S]
            )

    def load_wv():
        for k in range(8):
            nc.sync.dma_start(out=wv_t[k][:], in_=d["wv"][128 * k : 128 * (k + 1), :])

    def load_wo():
        for k in range(2):
            nc.sync.dma_start(out=wo_sb[k][:], in_=d["wo"][128 * k : 128 * (k + 1), :])

    # ---------------- emission helpers (closures alloc at pump time) -------
    def qk_chain(w_t, m, n, dst, bias_sb):
        """One [128,512] column chunk of a Q/K projection: 2 mm chunks + evac."""
        cell = {}

        def mk(k0):
            def go():
                if k0 == 0:
                    cell["ps"] = gpp.tile([128, 512], F32, name="ps", tag="gp")
                ps = cell["ps"]
                for k in range(k0, k0 + 4):
                    nc.tensor.matmul(
                        ps[:],
                        lhsT=w_t[k][:, 128 * m : 128 * (m + 1)],
                        rhs=xt[k][:, 512 * n : 512 * (n + 1)],
                        start=(k == 0),
                        stop=(k == 7),
                    )
            return go

        def evac():
            nc.vector.tensor_scalar_add(
                out=dst[m][:, 512 * n : 512 * (n + 1)],
                in0=cell["ps"][:],
                scalar1=bias_sb[:, m : m + 1],
            )

        return [mk(0), mk(4), evac]

    def v_chunk(sc):
        cell = {}

        def mk(k0):
            def go():
                if k0 == 0:
                    cell["ps"] = gpp.tile([128, GC], F32, name="psv", tag="gp")
                ps = cell["ps"]
                for k in range(k0, k0 + 4):
                    nc.tensor.matmul(
                        ps[:],
                        lhsT=xt[k][:, 128 * sc : 128 * (sc + 1)],
                        rhs=wv_t[k][:, :],
                        start=(k == 0),
                        stop=(k == 7),
                    )
            return go

        def evac():
            ps3 = cell["ps"][:].rearrange("p (h c) -> p h c", h=4)
            bv3 = bvrep[:].rearrange("p (h c) -> p h c", h=4)
            v3 = vstore[sc][:].rearrange("p (h c) -> p h c", h=4)
            nc.vector.tensor_add(v3[:, :, 0:64], ps3, bv3)

        return [mk(0), mk(4), evac]

    def rope_chunks(qg, which):
        """Rotary for local head 0 on query group qg of q or k (identity on
        cores g>0).

        rotate_half is a row permutation with signs: rot(q) = P q, computed
        on the PE with the tiny host-provided rotm ([64,64], +-1 entries) as
        the stationary operand; then dst = q*cos + rot*sin on DVE."""
        j = 0 if which == "q" else 1
        src = (qt[0], kt[0])[j]
        out = []
        for half in range(2):
            def go(src=src, j=j, half=half):
                sl = slice(QG * qg + 512 * half, QG * qg + 512 * (half + 1))
                dst = src[0:64, sl] if inplace_rope else (qtr, ktr)[j][:, sl]
                ps = gpp.tile([128, 512], F32, name="psr", tag="gp")
                nc.tensor.matmul(
                    ps[0:64, :], lhsT=rotm_sb[:], rhs=src[0:64, sl],
                    start=True, stop=True,
                )
                t1 = rp.tile([64, 512], BF16, name=f"t1{j}", tag="ropet1", bufs=2)
                nc.vector.tensor_mul(t1[:], ps[0:64, :], sin_sb[:, sl])
                nc.vector.tensor_mul(dst, src[0:64, sl], cos_sb[:, sl])
                nc.vector.tensor_add(dst, dst, t1[:])
            out.append(go)
        return out

    pts = {}

    def score_unit(i, hh, blk):
        qg, p = STREAMS[i]
        st = stp.tile([128, QG], F32, name="st", tag="st")
        rope = p == 0 and hh == 0 and not inplace_rope
        for half in range(2):
            csl = slice(QG * qg + 512 * half, QG * qg + 512 * (half + 1))
            k_ap = (
                ktr[:, 128 * blk : 128 * (blk + 1)]
                if rope
                else kt[p][64 * hh : 64 * (hh + 1), 128 * blk : 128 * (blk + 1)]
            )
            q_ap = qtr[:, csl] if rope else qt[p][64 * hh : 64 * (hh + 1), csl]
            nc.tensor.matmul(
                st[:, 512 * half : 512 * (half + 1)],
                lhsT=k_ap,
                rhs=q_ap,
                start=True,
                stop=True,
            )
        pt = ptp.tile([128, QG], BF16, name="pt", tag="pt", bufs=PT_BUFS)
        nc.scalar.activation(pt[:], st[:], EXP_FN, scale=0.125)
        pts[(i, hh, blk)] = pt

    def pv_chunks(i, hh):
        """O^T for (iteration i, head-in-pair hh): 2 chains of 16 mm in 4-mm
        chunks, then the fused normalize-into-otst closure."""
        qg, p = STREAMS[i]
        h = 2 * p + hh
        cell = {}
        chunks = []

        for half in range(2):
            for b0 in range(0, 16, 4):
                def go(half=half, b0=b0):
                    if b0 == 0:
                        cell[half] = otp.tile([128, 512], F32, name="ov", tag="ov")
                    ps = cell[half]
                    for blk in range(b0, b0 + 4):
                        nc.tensor.matmul(
                            ps[0:65, :],
                            lhsT=vstore[blk][:, 65 * h : 65 * h + 65],
                            rhs=pts[(i, hh, blk)][:, 512 * half : 512 * (half + 1)],
                            start=(blk == 0),
                            stop=(blk == 15),
                        )
                chunks.append(go)

        def norm():
            ot_un = nrm.tile([128, QG], BF16, name="ot_un", tag="ot_un", bufs=3)
            for half in range(2):
                nc.vector.tensor_copy(
                    ot_un[0:65, 512 * half : 512 * (half + 1)],
                    cell[half][0:65, :],
                )
            rrow = nrm.tile([1, QG], F32, name="rrow", tag="rrow", bufs=2)
            nc.vector.reciprocal(rrow[:], ot_un[64:65, :])
            scr = dscr.tile([1, QG], F32, name="scr", tag="scr")
            nc.sync.dma_start(out=scr[:], in_=rrow[:])
            bc = nrm.tile([128, QG], F32, name="bc", tag="bc", bufs=2)
            nc.sync.dma_start(out=bc[0:64, :], in_=scr[:].to_broadcast([64, QG]))
            qsl = slice(QG * qg, QG * (qg + 1))
            if hh == 0:
                nc.vector.tensor_mul(otst[p][0:64, qsl], ot_un[0:64, :], bc[0:64, :])
            else:
                # DVE cannot write partition base 64 from base-0 inputs;
                # stage through a tmp tile + sbuf->sbuf DMA.
                tmp = nrm.tile([64, QG], BF16, name="tmp", tag="tmp", bufs=2)
                nc.vector.tensor_mul(tmp[:], ot_un[0:64, :], bc[0:64, :])
                nc.sync.dma_start(out=otst[p][64:128, qsl], in_=tmp[:])

        chunks.append(norm)
        return chunks

    def wo_chunks(qg):
        """Natural-layout output projection for query group qg; 16 chunks."""
        chunks = []
        for sc in range(8):
            row0 = QG * qg + 128 * sc
            for n in range(2):
                def go(sc=sc, n=n, row0=row0):
                    ps = gpp.tile([128, 512], F32, name="wop", tag="gp")
                    for p in range(2):
                        nc.tensor.matmul(
                            ps[:],
                            lhsT=otst[p][:, row0 : row0 + 128],
                            rhs=wo_sb[p][:, 512 * n : 512 * (n + 1)],
                            start=(p == 0),
                            stop=(p == 1),
                        )
                    ob = nrm.tile([128, 512], BF16, name="ob", tag="ob", bufs=4)
                    nc.vector.tensor_copy(ob[:], ps[:])
                    nc.sync.dma_start(
                        out=d["out"][row0 : row0 + 128, 512 * n : 512 * (n + 1)],
                        in_=ob[:],
                    )
                chunks.append(go)
        return chunks

    # ---------------- emission schedule ------------------------------------
    STREAMS = [(0, 0), (1, 0), (0, 1), (1, 1)]  # (query group, head pair)

    # pre-stream: just enough for the first score units of stream 0
    for chunk in qk_chain(wq_t, 0, 0, qt, bq_sb):
        chunk()
    for chunk in qk_chain(wq_t, 0, 1, qt, bq_sb):
        chunk()
    for chunk in qk_chain(wk_t, 0, 0, kt, bk_sb):
        chunk()

    fillers = [deque() for _ in range(4)]
    # stream 0 (qg0, p0). Ordering constraints: ALL of k's rope (both query
    # groups) plus q's rope(qg0) must be emitted before this stream's h0
    # units (unit 16); q's rope(qg1) before stream 1's h0 units.
    f = fillers[0]
    f.append(load_xtb)
    f.append(load_wv)
    for n in (1, 2, 3):
        f.extend(qk_chain(wk_t, 0, n, kt, bk_sb))
    f.extend(rope_chunks(0, "k"))
    f.extend(rope_chunks(1, "k"))
    f.extend(rope_chunks(0, "q"))
    f.append(load_wo)
    for n in (2, 3):
        f.extend(qk_chain(wq_t, 0, n, qt, bq_sb))
    f.extend(rope_chunks(1, "q"))
    for sc in range(16):
        f.extend(v_chunk(sc))
    # stream 1 (qg1, p0)
    f = fillers[1]
    f.extend(pv_chunks(0, 1))
    f.extend(pv_chunks(0, 0))
    for n in (0, 1):
        f.extend(qk_chain(wq_t, 1, n, qt, bq_sb))
    for n in (0, 1):
        f.extend(qk_chain(wk_t, 1, n, kt, bk_sb))
    # stream 2 (qg0, p1): K m1 n23 first (needed by its own later units)
    f = fillers[2]
    for n in (2, 3):
        f.extend(qk_chain(wk_t, 1, n, kt, bk_sb))
    f.extend(pv_chunks(1, 1))
    f.extend(pv_chunks(1, 0))
    for n in (2, 3):
        f.extend(qk_chain(wq_t, 1, n, qt, bq_sb))
    # stream 3 (qg1, p1)
    f = fillers[3]
    f.extend(pv_chunks(2, 1))
    f.extend(pv_chunks(2, 0))
    f.extend(wo_chunks(0))

    carry = deque()
    for i in range(4):
        # cross-stream deps (projections, rope) must be emitted before this
        # stream's score units — drain leftovers fully at the boundary
        while carry:
            carry.popleft()()
        work = fillers[i]
        u = 0
        for hh in (1, 0):
            for blk in range(16):
                score_unit(i, hh, blk)
                u += 1
                left = 32 - u
                budget = 2 if left == 0 or len(work) <= 2 * left else 3
                while work and budget > 0:
                    work.popleft()()
                    budget -= 1
            if i == 3 and hh == 1:
                # PV(3, h1) can start as soon as its exps are done
                work.extend(pv_chunks(3, 1))
        carry = work

    # tail: drain leftovers, then PV(3, h0) + Wo(qg1)
    while carry:
        carry.popleft()()
    for chunk in pv_chunks(3, 0):
        chunk()
    for chunk in wo_chunks(1):
        chunk()


def build_nc(reps: int = 1):
    nc = bacc.Bacc(
        "TRN2", target_bir_lowering=False, debug=False, num_devices=N_CORES
    )
    d = {}
    d["xT"] = nc.dram_tensor("xT", [DIM, S], BF16, kind="ExternalInput").ap()
    for nm in ("wq", "wk", "wv"):
        d[nm] = nc.dram_tensor(nm, [DIM, GC], BF16, kind="ExternalInput").ap()
    for nm in ("bq2", "bk2"):
        d[nm] = nc.dram_tensor(nm, [128, 2], F32, kind="ExternalInput").ap()
    d["bvrow"] = nc.dram_tensor("bvrow", [1, GC], F32, kind="ExternalInput").ap()
    d["wo"] = nc.dram_tensor("wo", [GC, DIM], BF16, kind="ExternalInput").ap()
    d["cosT"] = nc.dram_tensor("cosT", [64, S], BF16, kind="ExternalInput").ap()
    d["sinT"] = nc.dram_tensor("sinT", [64, S], BF16, kind="ExternalInput").ap()
    d["rotm"] = nc.dram_tensor("rotm", [64, 64], BF16, kind="ExternalInput").ap()
    d["out"] = nc.dram_tensor("out", [S, DIM], BF16, kind="ExternalOutput").ap()

    inplace_rope = reps == 1
    with tile.TileContext(nc) as tc, ExitStack() as ctx:
        if reps == 1:
            _emit_body(nc, tc, ctx, d, inplace_rope)
        else:
            def body(_iv):
                with ExitStack() as inner:
                    _emit_body(nc, tc, inner, d, inplace_rope)

            with tc.For_i(0, reps, 1) as iv:
                body(iv)
    nc.compile()
    return nc


def shard_inputs(x, cos, sin, Wq, bq, Wk, bk, Wv, bv, Wo, bo):
    """Build the per-core input maps (host-side sharding + bf16 cast)."""
    import ml_dtypes

    bf16 = ml_dtypes.bfloat16
    x = np.asarray(x, np.float32)
    cos = np.asarray(cos, np.float32).reshape(S, 64)
    sin = np.asarray(sin, np.float32).reshape(S, 64)
    cosT = np.ascontiguousarray(cos.T).astype(bf16)
    sinT = np.ascontiguousarray(sin.T).astype(bf16)
    ones_cos = np.ones((64, S), bf16)
    zero_sin = np.zeros((64, S), bf16)
    xTs = [np.ascontiguousarray(x[b].T).astype(bf16) for b in range(B)]
    # rotate_half as a signed row permutation: rot = P q with
    # rot[2i] = -q[2i+1], rot[2i+1] = q[2i]; rotm[k, m] = P[m, k]
    rotm = np.zeros((64, 64), np.float32)
    for i in range(32):
        rotm[2 * i + 1, 2 * i] = -1.0
        rotm[2 * i, 2 * i + 1] = 1.0
    rotm = rotm.astype(bf16)

    in_maps = []
    for c in range(N_CORES):
        b, g = divmod(c, TP)
        sl = slice(GC * g, GC * (g + 1))
        m = {
            "xT": xTs[b],
            "wq": np.ascontiguousarray(np.asarray(Wq)[sl, :].T).astype(bf16),
            "wk": np.ascontiguousarray(np.asarray(Wk)[sl, :].T).astype(bf16),
            "wv": np.ascontiguousarray(np.asarray(Wv)[sl, :].T).astype(bf16),
            "bq2": np.ascontiguousarray(np.asarray(bq, np.float32)[sl].reshape(2, 128).T),
            "bk2": np.ascontiguousarray(np.asarray(bk, np.float32)[sl].reshape(2, 128).T),
            "bvrow": np.asarray(bv, np.float32)[sl].reshape(1, GC).copy(),
            "wo": np.ascontiguousarray(np.asarray(Wo)[:, sl].T).astype(bf16),
            "cosT": cosT if g == 0 else ones_cos,
            "sinT": sinT if g == 0 else zero_sin,
            "rotm": rotm,
        }
        in_maps.append(m)
    return in_maps


def unshard_output(results, bo):
    bo = np.asarray(bo, np.float32)
    out = np.empty((B, S, DIM), np.float32)
    for b in range(B):
        acc = results[TP * b]["out"].astype(np.float32).copy()
        for g in range(1, TP):
            acc += results[TP * b + g]["out"]
        out[b] = acc + bo
    return out


_NC_CACHE = {}


def get_nc(reps: int = 1):
    if reps not in _NC_CACHE:
        _NC_CACHE[reps] = build_nc(reps)
    return _NC_CACHE[reps]


def kernel(x, cos, sin, Wq, bq, Wk, bk, Wv, bv, Wo, bo, mask=None, _reps=1):
    nc = get_nc(_reps)
    in_maps = shard_inputs(x, cos, sin, Wq, bq, Wk, bk, Wv, bv, Wo, bo)
    res = run_bass_kernel_spmd(nc, in_maps, list(range(N_CORES)))
    return unshard_output(res.results, bo)


# revision 5
# speedup vs baseline: 1.1396x; 1.1396x over previous
"""DiT attention kernel for 8 Trainium2 NeuronCores (v2).

Sharding: tensor-parallel over head groups (4 groups of 4 heads) x
data-parallel over batch (2) = 8 cores. Each core computes, for its batch b
and head group g (heads 4g..4g+3):
    Q^T/K^T = (Wq/Wk slice)^T x^T in [gc, seq] layout (gc = 256 local dims)
    V in natural [seq, gc] layout (no PE transpose needed)
    partial rotary on global head 0 (cores with g==0; others get cos=1/sin=0)
    per head: S^T = K Q^T, P^T = exp(S^T/8), O^T = V_aug^T P^T where V_aug
    carries a ones column per head giving softmax denominators as an extra
    output row; normalize via reciprocal + sbuf broadcast-DMA; Wo partial
    product in natural [seq, dim] layout, summed on host across head groups.

Everything flows through the PE in bf16 (fp32 PSUM accumulation); inputs are
cast host-side. Emission is manually software-pipelined: exp (ACT) is the
pace-setter, all other PE work is interleaved between score matmuls in small
chunks to avoid head-of-line blocking.

vstore column layout per key block tile [128, 260]: head h occupies cols
[65h, 65h+64) with a ones column at 65h+64, so each head's PV output lands
at partitions 0..64 with the softmax denominator in row 64. Odd heads'
normalized output is staged through a tmp tile and DMA-shifted to otst rows
64..128 (DVE cannot write partition base 64 from base-0 inputs).
"""

import os
import sys

if "/opt/trn_rl_repo" not in sys.path:
    sys.path.insert(0, "/opt/trn_rl_repo")



from collections import deque
from contextlib import ExitStack

import numpy as np

import concourse.bass as bass  # noqa: F401  (bass must import before bacc)
import concourse.mybir as mybir
import concourse.tile as tile
from concourse import bacc
from concourse.bass_utils import run_bass_kernel_spmd

F32 = mybir.dt.float32
F32R = mybir.dt.float32r
BF16 = mybir.dt.bfloat16
EXP_FN = mybir.ActivationFunctionType.Exp

B, S, DIM, HEADS, HEAD_DIM = 2, 2048, 1024, 16, 64
N_CORES = 8
TP = 4                      # head groups
GH = HEADS // TP            # heads per core (4)
GC = GH * HEAD_DIM          # cols per core slice (256)
QG = 1024                   # query group width
PT_BUFS = 38                # pts ring depth (~2 iterations + slack)


def _emit_body(nc, tc, ctx, d, inplace_rope=True):
    consts = ctx.enter_context(tc.tile_pool(name="consts", bufs=1))

    ones4 = consts.tile([128, 4], F32, name="ones4", tag="ones4")
    nc.vector.memset(ones4[:], 1.0)

    bq_sb = consts.tile([128, 2], F32, name="bq", tag="bq")
    bk_sb = consts.tile([128, 2], F32, name="bk", tag="bk")
    bvrep = consts.tile([128, GC], F32, name="bvrep", tag="bvrep")
    nc.sync.dma_start(out=bq_sb[:], in_=d["bq2"][:, :])
    nc.sync.dma_start(out=bk_sb[:], in_=d["bk2"][:, :])
    nc.sync.dma_start(out=bvrep[:], in_=d["bvrow"][:].to_broadcast([128, GC]))

    qt = [consts.tile([128, S], BF16, name=f"qt{i}", tag=f"qt{i}") for i in range(2)]
    kt = [consts.tile([128, S], BF16, name=f"kt{i}", tag=f"kt{i}") for i in range(2)]
    if not inplace_rope:
        qtr = consts.tile([64, S], BF16, name="qtr", tag="qtr")
        ktr = consts.tile([64, S], BF16, name="ktr", tag="ktr")
    vstore = [consts.tile([128, 260], BF16, name=f"vs{i}", tag=f"vs{i}") for i in range(16)]
    # normalized O^T, heads stacked per pair p: rows 0:64 head 2p, 64:128 head 2p+1
    otst = [consts.tile([128, S], BF16, name=f"ot{i}", tag=f"ot{i}") for i in range(2)]

    cos_sb = consts.tile([64, S], BF16, name="cos", tag="cos")
    sin_sb = consts.tile([64, S], BF16, name="sin", tag="sin")
    rotm_sb = consts.tile([64, 64], BF16, name="rotm", tag="rotm")
    wo_sb = [consts.tile([128, DIM], BF16, name=f"wo{k}", tag=f"wo{k}") for k in range(2)]

    # ones columns of vstore: col 65h+64 for each head h
    for sc in range(16):
        v3 = vstore[sc][:].rearrange("p (h c) -> p h c", h=4)
        nc.vector.tensor_copy(v3[:, :, 64:65], ones4[:].rearrange("p (h c) -> p h c", h=4))

    xw = ctx.enter_context(tc.tile_pool(name="xw", bufs=1))
    wsp = ctx.enter_context(tc.tile_pool(name="wstream", bufs=24))
    ptp = ctx.enter_context(tc.tile_pool(name="ptp", bufs=PT_BUFS))
    nrm = ctx.enter_context(tc.tile_pool(name="nrm", bufs=3))
    rp = ctx.enter_context(tc.tile_pool(name="rope", bufs=2))
    dscr = ctx.enter_context(tc.tile_pool(name="dscr", bufs=4, space="DRAM"))

    # psum: st 2x2 banks + gp 2x1 + otp 2x1 = 8 banks
    stp = ctx.enter_context(tc.tile_pool(name="stp", bufs=2, space="PSUM"))
    gpp = ctx.enter_context(tc.tile_pool(name="gpp", bufs=2, space="PSUM"))
    otp = ctx.enter_context(tc.tile_pool(name="otp", bufs=2, space="PSUM"))

    xt = [xw.tile([128, S], BF16, name=f"xt{k}", tag=f"xt{k}") for k in range(8)]
    wq_t = [wsp.tile([128, GC], BF16, name="w", tag="w") for _ in range(8)]
    wk_t = [wsp.tile([128, GC], BF16, name="w", tag="w") for _ in range(8)]
    wv_t = [wsp.tile([128, GC], BF16, name="w", tag="w") for _ in range(8)]

    # input DMAs, ordered by need
    for k in range(8):
        nc.sync.dma_start(out=wq_t[k][:], in_=d["wq"][128 * k : 128 * (k + 1), :])
        nc.sync.dma_start(out=xt[k][:, 0:QG], in_=d["xT"][128 * k : 128 * (k + 1), 0:QG])
    for k in range(8):
        nc.sync.dma_start(out=wk_t[k][:], in_=d["wk"][128 * k : 128 * (k + 1), :])
    nc.sync.dma_start(out=cos_sb[:], in_=d["cosT"][:, :])
    nc.sync.dma_start(out=sin_sb[:], in_=d["sinT"][:, :])
    nc.sync.dma_start(out=rotm_sb[:], in_=d["rotm"][:, :])

    def load_xtb():
        # second halves of x^T; deferred past the pre-stream chains so
        # whole-tile deps don't gate them on these DMAs
        for k in range(8):
            nc.sync.dma_start(
                out=xt[k][:, QG:S], in_=d["xT"][128 * k : 128 * (k + 1), QG:S]
            )

    def load_wv():
        for k in range(8):
            nc.sync.dma_start(out=wv_t[k][:], in_=d["wv"][128 * k : 128 * (k + 1), :])

    def load_wo():
        for k in range(2):
            nc.sync.dma_start(out=wo_sb[k][:], in_=d["wo"][128 * k : 128 * (k + 1), :])

    # ---------------- emission helpers (closures alloc at pump time) -------
    def qk_chain(w_t, m, n, dst, bias_sb):
        """One [128,512] column chunk of a Q/K projection: 2 mm chunks + evac."""
        cell = {}

        def mk(k0):
            def go():
                if k0 == 0:
                    cell["ps"] = gpp.tile([128, 512], F32, name="ps", tag="gp")
                ps = cell["ps"]
                for k in range(k0, k0 + 4):
                    nc.tensor.matmul(
                        ps[:],
                        lhsT=w_t[k][:, 128 * m : 128 * (m + 1)],
                        rhs=xt[k][:, 512 * n : 512 * (n + 1)],
                        start=(k == 0),
                        stop=(k == 7),
                    )
            return go

        def evac():
            nc.vector.tensor_scalar_add(
                out=dst[m][:, 512 * n : 512 * (n + 1)],
                in0=cell["ps"][:],
                scalar1=bias_sb[:, m : m + 1],
            )

        return [mk(0), mk(4), evac]

    def v_chunk(sc):
        cell = {}

        def mk(k0):
            def go():
                if k0 == 0:
                    cell["ps"] = gpp.tile([128, GC], F32, name="psv", tag="gp")
                ps = cell["ps"]
                for k in range(k0, k0 + 4):
                    nc.tensor.matmul(
                        ps[:],
                        lhsT=xt[k][:, 128 * sc : 128 * (sc + 1)],
                        rhs=wv_t[k][:, :],
                        start=(k == 0),
                        stop=(k == 7),
                    )
            return go

        def evac():
            ps3 = cell["ps"][:].rearrange("p (h c) -> p h c", h=4)
            bv3 = bvrep[:].rearrange("p (h c) -> p h c", h=4)
            v3 = vstore[sc][:].rearrange("p (h c) -> p h c", h=4)
            nc.vector.tensor_add(v3[:, :, 0:64], ps3, bv3)

        return [mk(0), mk(4), evac]

    def rope_chunks(qg, which):
        """Rotary for local head 0 on query group qg of q or k (identity on
        cores g>0).

        rotate_half is a row permutation with signs: rot(q) = P q, computed
        on the PE with the tiny host-provided rotm ([64,64], +-1 entries) as
        the stationary operand; then dst = q*cos + rot*sin on DVE."""
        j = 0 if which == "q" else 1
        src = (qt[0], kt[0])[j]
        out = []
        for half in range(2):
            def go(src=src, j=j, half=half):
                sl = slice(QG * qg + 512 * half, QG * qg + 512 * (half + 1))
                dst = src[0:64, sl] if inplace_rope else (qtr, ktr)[j][:, sl]
                ps = gpp.tile([128, 512], F32, name="psr", tag="gp")
                nc.tensor.matmul(
                    ps[0:64, :], lhsT=rotm_sb[:], rhs=src[0:64, sl],
                    start=True, stop=True,
                )
                t1 = rp.tile([64, 512], BF16, name=f"t1{j}", tag="ropet1", bufs=2)
                nc.vector.tensor_mul(t1[:], ps[0:64, :], sin_sb[:, sl])
                nc.vector.tensor_mul(dst, src[0:64, sl], cos_sb[:, sl])
                nc.vector.tensor_add(dst, dst, t1[:])
            out.append(go)
        return out

    pts = {}

    def score_unit(i, hh, blk):
        qg, p = STREAMS[i]
        st = stp.tile([128, QG], F32, name="st", tag="st")
        rope = p == 0 and hh == 0 and not inplace_rope
        for half in range(2):
            csl = slice(QG * qg + 512 * half, QG * qg + 512 * (half + 1))
            k_ap = (
                ktr[:, 128 * blk : 128 * (blk + 1)]
                if rope
                else kt[p][64 * hh : 64 * (hh + 1), 128 * blk : 128 * (blk + 1)]
            )
            q_ap = qtr[:, csl] if rope else qt[p][64 * hh : 64 * (hh + 1), csl]
            nc.tensor.matmul(
                st[:, 512 * half : 512 * (half + 1)],
                lhsT=k_ap,
                rhs=q_ap,
                start=True,
                stop=True,
            )
        pt = ptp.tile([128, QG], BF16, name="pt", tag="pt", bufs=PT_BUFS)
        nc.scalar.activation(pt[:], st[:], EXP_FN, scale=0.125)
        pts[(i, hh, blk)] = pt

    def pv_chunks(i, hh):
        """O^T for (iteration i, head-in-pair hh): 2 chains of 16 mm in 4-mm
        chunks, then the fused normalize-into-otst closure."""
        qg, p = STREAMS[i]
        h = 2 * p + hh
        cell = {}
        chunks = []

        for half in range(2):
            for b0 in range(0, 16, 4):
                def go(half=half, b0=b0):
                    if b0 == 0:
                        cell[half] = otp.tile([128, 512], F32, name="ov", tag="ov")
                    ps = cell[half]
                    for blk in range(b0, b0 + 4):
                        nc.tensor.matmul(
                            ps[0:65, :],
                            lhsT=vstore[blk][:, 65 * h : 65 * h + 65],
                            rhs=pts[(i, hh, blk)][:, 512 * half : 512 * (half + 1)],
                            start=(blk == 0),
                            stop=(blk == 15),
                        )
                chunks.append(go)

        def norm():
            # stage O^T + denominator row to SBUF (frees the PV psum ring
            # quickly), reciprocal, partition-broadcast via a DRAM round trip,
            # then normalize into otst.
            ot_un = nrm.tile([128, QG], BF16, name="ot_un", tag="ot_un", bufs=3)
            for half in range(2):
                nc.vector.tensor_copy(
                    ot_un[0:65, 512 * half : 512 * (half + 1)],
                    cell[half][0:65, :],
                )
            rrow = nrm.tile([1, QG], F32, name="rrow", tag="rrow", bufs=2)
            nc.vector.reciprocal(rrow[:], ot_un[64:65, :])
            scr = dscr.tile([1, QG], F32, name="scr", tag="scr")
            nc.sync.dma_start(out=scr[:], in_=rrow[:])
            bc = nrm.tile([128, QG], F32, name="bc", tag="bc", bufs=2)
            nc.sync.dma_start(out=bc[0:64, :], in_=scr[:].to_broadcast([64, QG]))
            qsl = slice(QG * qg, QG * (qg + 1))
            if hh == 0:
                nc.vector.tensor_mul(otst[p][0:64, qsl], ot_un[0:64, :], bc[0:64, :])
            else:
                # DVE cannot write partition base 64 from base-0 inputs;
                # stage through a tmp tile + sbuf->sbuf DMA.
                tmp = nrm.tile([64, QG], BF16, name="tmp", tag="tmp", bufs=2)
                nc.vector.tensor_mul(tmp[:], ot_un[0:64, :], bc[0:64, :])
                nc.sync.dma_start(out=otst[p][64:128, qsl], in_=tmp[:])

        chunks.append(norm)
        return chunks

    def wo_chunks(qg):
        """Natural-layout output projection for query group qg; 16 chunks."""
        chunks = []
        for sc in range(8):
            row0 = QG * qg + 128 * sc
            for n in range(2):
                def go(sc=sc, n=n, row0=row0):
                    ps = gpp.tile([128, 512], F32, name="wop", tag="gp")
                    for p in range(2):
                        nc.tensor.matmul(
                            ps[:],
                            lhsT=otst[p][:, row0 : row0 + 128],
                            rhs=wo_sb[p][:, 512 * n : 512 * (n + 1)],
                            start=(p == 0),
                            stop=(p == 1),
                        )
                    ob = nrm.tile([128, 512], BF16, name="ob", tag="ob", bufs=4)
                    nc.vector.tensor_copy(ob[:], ps[:])
                    nc.sync.dma_start(
                        out=d["out"][row0 : row0 + 128, 512 * n : 512 * (n + 1)],
                        in_=ob[:],
                    )
                chunks.append(go)
        return chunks

    # ---------------- emission schedule ------------------------------------
    STREAMS = [(0, 0), (1, 0), (0, 1), (1, 1)]  # (query group, head pair)

    # pre-stream: just enough for the first score units of stream 0
    for chunk in qk_chain(wq_t, 0, 0, qt, bq_sb):
        chunk()
    for chunk in qk_chain(wq_t, 0, 1, qt, bq_sb):
        chunk()
    for chunk in qk_chain(wk_t, 0, 0, kt, bk_sb):
        chunk()

    fillers = [deque() for _ in range(4)]
    # stream 0 (qg0, p0). Ordering constraints: ALL of k's rope (both query
    # groups) plus q's rope(qg0) must be emitted before this stream's h0
    # units (unit 16); q's rope(qg1) before stream 1's h0 units.
    f = fillers[0]
    f.append(load_xtb)
    f.append(load_wv)
    for n in (1, 2, 3):
        f.extend(qk_chain(wk_t, 0, n, kt, bk_sb))
    f.extend(rope_chunks(0, "k"))
    f.extend(rope_chunks(1, "k"))
    f.extend(rope_chunks(0, "q"))
    f.append(load_wo)
    for n in (2, 3):
        f.extend(qk_chain(wq_t, 0, n, qt, bq_sb))
    f.extend(rope_chunks(1, "q"))
    for sc in range(16):
        f.extend(v_chunk(sc))
    # stream 1 (qg1, p0)
    f = fillers[1]
    f.extend(pv_chunks(0, 1))
    f.extend(pv_chunks(0, 0))
    for n in (0, 1):
        f.extend(qk_chain(wq_t, 1, n, qt, bq_sb))
    for n in (0, 1):
        f.extend(qk_chain(wk_t, 1, n, kt, bk_sb))
    # stream 2 (qg0, p1): K m1 n23 first (needed by its own later units)
    f = fillers[2]
    for n in (2, 3):
        f.extend(qk_chain(wk_t, 1, n, kt, bk_sb))
    f.extend(pv_chunks(1, 1))
    f.extend(pv_chunks(1, 0))
    for n in (2, 3):
        f.extend(qk_chain(wq_t, 1, n, qt, bq_sb))
    # stream 3 (qg1, p1)
    f = fillers[3]
    f.extend(pv_chunks(2, 1))
    f.extend(pv_chunks(2, 0))
    f.extend(wo_chunks(0))

    carry = deque()
    for i in range(4):
        # cross-stream deps (projections, rope) must be emitted before this
        # stream's score units — drain leftovers fully at the boundary
        while carry:
            carry.popleft()()
        work = fillers[i]
        u = 0
        for hh in (1, 0):
            for blk in range(16):
                score_unit(i, hh, blk)
                u += 1
                left = max(32 - u, 1)
                budget = max(2, min(4, -(-len(work) // left)))
                while work and budget > 0:
                    work.popleft()()
                    budget -= 1
            if i == 3 and hh == 1:
                # PV(3, h1) can start as soon as its exps are done
                work.extend(pv_chunks(3, 1))
        carry = work

    # tail: drain leftovers, then PV(3, h0) + Wo(qg1)
    while carry:
        carry.popleft()()
    for chunk in pv_chunks(3, 0):
        chunk()
    for chunk in wo_chunks(1):
        chunk()


def build_nc(reps: int = 1):
    nc = bacc.Bacc(
        "TRN2", target_bir_lowering=False, debug=False, num_devices=N_CORES
    )
    d = {}
    d["xT"] = nc.dram_tensor("xT", [DIM, S], BF16, kind="ExternalInput").ap()
    for nm in ("wq", "wk", "wv"):
        d[nm] = nc.dram_tensor(nm, [DIM, GC], BF16, kind="ExternalInput").ap()
    for nm in ("bq2", "bk2"):
        d[nm] = nc.dram_tensor(nm, [128, 2], F32, kind="ExternalInput").ap()
    d["bvrow"] = nc.dram_tensor("bvrow", [1, GC], F32, kind="ExternalInput").ap()
    d["wo"] = nc.dram_tensor("wo", [GC, DIM], BF16, kind="ExternalInput").ap()
    d["cosT"] = nc.dram_tensor("cosT", [64, S], BF16, kind="ExternalInput").ap()
    d["sinT"] = nc.dram_tensor("sinT", [64, S], BF16, kind="ExternalInput").ap()
    d["rotm"] = nc.dram_tensor("rotm", [64, 64], BF16, kind="ExternalInput").ap()
    d["out"] = nc.dram_tensor("out", [S, DIM], BF16, kind="ExternalOutput").ap()

    inplace_rope = reps == 1
    with tile.TileContext(nc) as tc, ExitStack() as ctx:
        if reps == 1:
            _emit_body(nc, tc, ctx, d, inplace_rope)
        else:
            def body(_iv):
                with ExitStack() as inner:
                    _emit_body(nc, tc, inner, d, inplace_rope)

            with tc.For_i(0, reps, 1) as iv:
                body(iv)
    nc.compile()
    return nc


def shard_inputs(x, cos, sin, Wq, bq, Wk, bk, Wv, bv, Wo, bo):
    """Build the per-core input maps (host-side sharding + bf16 cast)."""
    import ml_dtypes

    bf16 = ml_dtypes.bfloat16
    x = np.asarray(x, np.float32)
    cos = np.asarray(cos, np.float32).reshape(S, 64)
    sin = np.asarray(sin, np.float32).reshape(S, 64)
    cosT = np.ascontiguousarray(cos.T).astype(bf16)
    sinT = np.ascontiguousarray(sin.T).astype(bf16)
    ones_cos = np.ones((64, S), bf16)
    zero_sin = np.zeros((64, S), bf16)
    xTs = [np.ascontiguousarray(x[b].T).astype(bf16) for b in range(B)]
    # rotate_half as a signed row permutation: rot = P q with
    # rot[2i] = -q[2i+1], rot[2i+1] = q[2i]; rotm[k, m] = P[m, k]
    rotm = np.zeros((64, 64), np.float32)
    for i in range(32):
        rotm[2 * i + 1, 2 * i] = -1.0
        rotm[2 * i, 2 * i + 1] = 1.0
    rotm = rotm.astype(bf16)

    in_maps = []
    for c in range(N_CORES):
        b, g = divmod(c, TP)
        sl = slice(GC * g, GC * (g + 1))
        m = {
            "xT": xTs[b],
            "wq": np.ascontiguousarray(np.asarray(Wq)[sl, :].T).astype(bf16),
            "wk": np.ascontiguousarray(np.asarray(Wk)[sl, :].T).astype(bf16),
            "wv": np.ascontiguousarray(np.asarray(Wv)[sl, :].T).astype(bf16),
            "bq2": np.ascontiguousarray(np.asarray(bq, np.float32)[sl].reshape(2, 128).T),
            "bk2": np.ascontiguousarray(np.asarray(bk, np.float32)[sl].reshape(2, 128).T),
            "bvrow": np.asarray(bv, np.float32)[sl].reshape(1, GC).copy(),
            "wo": np.ascontiguousarray(np.asarray(Wo)[:, sl].T).astype(bf16),
            "cosT": cosT if g == 0 else ones_cos,
            "sinT": sinT if g == 0 else zero_sin,
            "rotm": rotm,
        }
        in_maps.append(m)
    return in_maps


def unshard_output(results, bo):
    bo = np.asarray(bo, np.float32)
    out = np.empty((B, S, DIM), np.float32)
    for b in range(B):
        acc = results[TP * b]["out"].astype(np.float32).copy()
        for g in range(1, TP):
            acc += results[TP * b + g]["out"]
        out[b] = acc + bo
    return out


_NC_CACHE = {}


def get_nc(reps: int = 1):
    if reps not in _NC_CACHE:
        _NC_CACHE[reps] = build_nc(reps)
    return _NC_CACHE[reps]


def kernel(x, cos, sin, Wq, bq, Wk, bk, Wv, bv, Wo, bo, mask=None, _reps=1):
    nc = get_nc(_reps)
    in_maps = shard_inputs(x, cos, sin, Wq, bq, Wk, bk, Wv, bv, Wo, bo)
    res = run_bass_kernel_spmd(nc, in_maps, list(range(N_CORES)))
    return unshard_output(res.results, bo)
